# revision 1
# baseline (speedup 1.0000x reference)
"""Trainium2 Bass kernel for nn_RNN: h_t = x_t @ W + h_{t-1} @ R (linear RNN).

Full shapes: sequences [64, 512, 1024], kernel [1024, 1024],
recurrent_kernel [1024, 1024], h0 [64, 1024] -> out [64, 512, 1024].

Sharding: data-parallel over batch across 8 cores (8 sequences/core).

Per-core algorithm (blocked scan, K=16 block length, NB=32 blocks):
  r-index = blk*8 + batch  (256 scan lanes)
  Phase X: xproj: XpT[u, j, r] = W.T @ X.T   (PE-transpose X tiles on chip)
  Phase S: R^16 by repeated squaring (pair chain to keep lhsT transposed)
  Phase A: zero-init within-block scans, batched over all 256 lanes
           (transposed state ST[u, r]); keeps only block-final partials P
  Phase B: boundary scan across 32 blocks with R^16 (thin, sequential)
  Phase C: re-scan with true block-initial states; emits all outputs
           (PE-transpose back to natural [row, u] layout for DMA out)

All matmuls in fp32r (FP22 mantissa, fp32 accumulate).
"""
import sys
import numpy as np

sys.path.insert(0, "/opt/trn_rl_repo")

try:  # persistent jit cache: repeated kernel() invocations skip recompile
    import jax
    import os as _os
    _cache = _os.environ.get("JAX_COMPILATION_CACHE_DIR", "/tmp/jaxcache_rnn")
    _os.makedirs(_cache, exist_ok=True)
    jax.config.update("jax_compilation_cache_dir", _cache)
except Exception:
    pass

import concourse.bass as bass  # noqa: E402
import concourse.tile as tile  # noqa: E402
from concourse import bacc, mybir  # noqa: E402
from concourse.masks import make_identity  # noqa: E402

FP32 = mybir.dt.float32
FP32R = mybir.dt.float32r
BF16 = mybir.dt.bfloat16

NCORES = 8
B, T, F, U = 64, 512, 1024, 1024
BC = B // NCORES          # batch per core = 8
K = 16                    # block length
NB = T // K               # 32 blocks
R_LANES = NB * BC         # 256 scan lanes
P = 128                   # partitions
FC = F // P               # 8 f-chunks
UC = U // P               # 8 u-chunks


def f32r(ap):
    return ap.bitcast(FP32R)


PHASE_MARKS = []


def _mark(nc, name):
    PHASE_MARKS.append((name, nc.next_id()))


def build_nc():
    PHASE_MARKS.clear()
    nc = bacc.Bacc("TRN2", target_bir_lowering=False, debug=False,
                   num_devices=NCORES)

    seq = nc.dram_tensor("seq", [BC, T, F], FP32, kind="ExternalInput").ap()
    w_in = nc.dram_tensor("w", [F, U], FP32, kind="ExternalInput").ap()
    r_in = nc.dram_tensor("r", [U, U], FP32, kind="ExternalInput").ap()
    h0_in = nc.dram_tensor("h0", [BC, U], FP32, kind="ExternalInput").ap()
    reps_in = nc.dram_tensor("reps", [1, 1], mybir.dt.int32,
                             kind="ExternalInput").ap()
    out = nc.dram_tensor("out", [BC, T, U], FP32, kind="ExternalOutput").ap()
    # xproj scratch, transposed layout [u, j, r]
    xp_d = nc.dram_tensor("xp_scratch", [U, K, R_LANES], FP32).ap()
    # output bounce scratch (bf16, transposed [u, j, r]); read back via
    # hardware DMA-transpose into natural [row, u] layout
    osc_d = nc.dram_tensor("out_scratch", [U, K, R_LANES], BF16).ap()

    # DRAM views
    # seq rows indexed by (blk, b): seq4[blk, b, t_in_block, f]
    seq4 = seq.rearrange("b (nb k) f -> nb b k f", k=K)
    xp_v = xp_d.rearrange("(m p) j r -> p m j r", p=P)       # [128, 8, 16, 256]
    osc_v = osc_d.rearrange("(m p) j r -> p m j r", p=P)
    osc_flat = osc_d.rearrange("u j r -> u (j r)")           # [1024, 4096]
    out4 = out.rearrange("b (nb k) u -> nb b k u", k=K)      # [32, 8, 16, 1024]

    with tile.TileContext(nc) as tc:
        def _body(_it=None):
            with (
                tc.tile_pool(name="consts", bufs=1) as consts,
                tc.tile_pool(name="mats", bufs=1) as mats,
                tc.tile_pool(name="psA", bufs=4, space="PSUM") as psA,
                tc.tile_pool(name="psT", bufs=4, space="PSUM") as psT,
            ):
                id128 = consts.tile([P, P], FP32)
                make_identity(nc, id128)
                id128b = consts.tile([P, P], BF16)
                make_identity(nc, id128b)
                id8 = consts.tile([BC, BC], FP32)
                make_identity(nc, id8)

                # ---- load W, R ----------------------------------------------
                # layout [p, kchunk, col]: tile[p, k, c] = M[k*128+p, c]
                # mats pool has 4 rotating 32KB/partition slots: r, s0, s1, s2.
                # W shares slot s2 (dead after xproj, before s2's first write).
                w_sb = mats.tile([P, FC, U], BF16, tag="s2")
                r_sb = mats.tile([P, UC, U], BF16, tag="r")
                nc.gpsimd.dma_start(
                    out=w_sb, in_=w_in.rearrange("(k p) u -> p k u", p=P))
                nc.gpsimd.dma_start(
                    out=r_sb, in_=r_in.rearrange("(k p) u -> p k u", p=P))

                # ---- Phase X: xproj -> xp_scratch ---------------------------
                # chunk jj covers j in {2jj, 2jj+1}; 512 w-rows per chunk:
                # w = j2*256 + blk*8 + b
                with (
                    tc.tile_pool(name="xnat", bufs=2) as xnat_p,
                    tc.tile_pool(name="xt", bufs=1) as xt_p,
                    tc.tile_pool(name="xpo", bufs=2) as xpo_p,
                ):
                    _mark(nc, "xproj")
                    for jj in range(8):
                        xt = xt_p.tile([P, FC, 512], BF16, tag="xt")
                        for s in range(4):
                            j2, half = s // 2, s % 2
                            tval = 2 * jj + j2
                            xn = xnat_p.tile([P, F], FP32, tag="xn")
                            nc.sync.dma_start(
                                out=xn,
                                in_=seq4[half * 16:(half + 1) * 16, :, tval, :],
                            )
                            for c in range(FC):
                                pt = psT.tile([P, P], FP32)
                                nc.tensor.transpose(pt, xn[:, c * P:(c + 1) * P], id128)
                                nc.vector.tensor_copy(xt[:, c, s * P:(s + 1) * P], pt)
                        for m in range(UC):
                            ps = psA.tile([P, 512], FP32)
                            for k in range(FC):
                                nc.tensor.matmul(
                                    ps, w_sb[:, k, m * P:(m + 1) * P],
                                    xt[:, k, :],
                                    start=(k == 0), stop=(k == FC - 1),
                                )
                            xo = xpo_p.tile([P, 512], FP32, tag="xo")
                            nc.scalar.copy(xo, ps)
                            nc.sync.dma_start(
                                out=xp_v[:, m, 2 * jj:2 * jj + 2, :],
                                in_=xo.rearrange("p (j r) -> p j r", j=2),
                            )

                # ---- Phase S: R^16 via squarings ----------------------------
                # pair chain keeps (M, M.T) so lhsT is always available
                def gemm1024(dst, lhsT_t, rhs_t):
                    # dst[p, mchunk, u] = (lhsT_t.T @ rhs_t) tiled
                    for m in range(UC):
                        for n in range(2):
                            ps = psA.tile([P, 512], FP32)
                            for k in range(UC):
                                nc.tensor.matmul(
                                    ps, lhsT_t[:, k, m * P:(m + 1) * P],
                                    rhs_t[:, k, n * 512:(n + 1) * 512],
                                    start=(k == 0), stop=(k == UC - 1),
                                )
                            nc.scalar.copy(dst[:, m, n * 512:(n + 1) * 512], ps)

                def transpose1024(dst, src):
                    # dst = src.T (both [P, UC, U] tiled layouts)
                    for kk in range(UC):
                        for cc in range(UC):
                            pt = psT.tile([P, P], BF16)
                            nc.tensor.transpose(
                                pt, src[:, cc, kk * P:(kk + 1) * P],
                                id128b)
                            nc.vector.tensor_copy(
                                dst[:, kk, cc * P:(cc + 1) * P], pt)

                # squaring chain with 3 rotating slots:
                #   s0 holds the transpose, s1/s2 alternate squared outputs
                _mark(nc, "squar")
                rt_t = mats.tile([P, UC, U], BF16, tag="s0")
                transpose1024(rt_t, r_sb)
                r2 = mats.tile([P, UC, U], BF16, tag="s1")
                gemm1024(r2, rt_t, r_sb)                  # R^2
                r2t = mats.tile([P, UC, U], BF16, tag="s0")
                transpose1024(r2t, r2)
                r4 = mats.tile([P, UC, U], BF16, tag="s2")
                gemm1024(r4, r2t, r2)                     # R^4
                r4t = mats.tile([P, UC, U], BF16, tag="s0")
                transpose1024(r4t, r4)
                r8 = mats.tile([P, UC, U], BF16, tag="s1")
                gemm1024(r8, r4t, r4)                     # R^8
                r8t = mats.tile([P, UC, U], BF16, tag="s0")
                transpose1024(r8t, r8)
                r16 = mats.tile([P, UC, U], BF16, tag="s2")
                gemm1024(r16, r8t, r8)                    # R^16

                # ---- Phase A: zero-init batched scan ------------------------
                with (
                    tc.tile_pool(name="st", bufs=2) as st_p,
                    tc.tile_pool(name="xps", bufs=2) as xps_p,
                    tc.tile_pool(name="psb", bufs=1) as psb_p,
                ):
                    _mark(nc, "A")
                    st_prev = st_p.tile([P, UC, R_LANES], BF16, tag="st")
                    xps0 = xps_p.tile([P, UC, R_LANES], FP32, tag="xps")
                    nc.sync.dma_start(out=xps0, in_=xp_v[:, :, 0, :])
                    nc.scalar.copy(st_prev, xps0)
                    for j in range(1, K):
                        xps = xps_p.tile([P, UC, R_LANES], FP32, tag="xps")
                        nc.sync.dma_start(out=xps, in_=xp_v[:, :, j, :])
                        st_new = st_p.tile([P, UC, R_LANES], BF16, tag="st")
                        for m in range(UC):
                            ps = psA.tile([P, R_LANES], FP32)
                            for k in range(UC):
                                nc.tensor.matmul(
                                    ps, r_sb[:, k, m * P:(m + 1) * P],
                                    st_prev[:, k, :],
                                    start=(k == 0), stop=(k == UC - 1),
                                )
                            nc.vector.tensor_add(st_new[:, m, :], ps, xps[:, m, :])
                        st_prev = st_new
                    psb = psb_p.tile([P, UC, R_LANES], FP32)
                    nc.scalar.copy(psb, st_prev)

                    # ---- Phase B: boundary scan (32 blocks, thin) -----------
                    with (
                        tc.tile_pool(name="hbt", bufs=2) as hbt_p,
                        tc.tile_pool(name="ci", bufs=1) as ci_p,
                        tc.tile_pool(name="mn", bufs=2) as mn_p,
                    ):
                        _mark(nc, "B")
                        ci = ci_p.tile([P, UC, R_LANES], BF16)
                        h0sb = mn_p.tile([BC, U], FP32, tag="h0")
                        nc.sync.dma_start(out=h0sb, in_=h0_in)
                        hbt = hbt_p.tile([P, UC, BC], BF16, tag="hbt")
                        for c in range(UC):
                            pt = psT.tile([P, BC], FP32)
                            nc.tensor.transpose(pt, h0sb[:, c * P:(c + 1) * P], id8)
                            nc.scalar.copy(hbt[:, c, :], pt)
                            nc.scalar.copy(ci[:, c, 0:BC], pt)
                        for b in range(NB):
                            mn = mn_p.tile([BC, U], FP32, tag="mn")
                            for n in range(2):
                                ps = psA.tile([BC, 512], FP32)
                                for k in range(UC):
                                    nc.tensor.matmul(
                                        ps, hbt[:, k, :],
                                        r16[:, k, n * 512:(n + 1) * 512],
                                        start=(k == 0), stop=(k == UC - 1),
                                    )
                                nc.scalar.copy(mn[:, n * 512:(n + 1) * 512], ps)
                            hbt_n = hbt_p.tile([P, UC, BC], BF16, tag="hbt")
                            for c in range(UC):
                                pt = psT.tile([P, BC], FP32)
                                nc.tensor.transpose(pt, mn[:, c * P:(c + 1) * P], id8)
                                nc.vector.tensor_add(
                                    hbt_n[:, c, :], pt, psb[:, c, b * BC:(b + 1) * BC])
                                if b < NB - 1:
                                    nc.scalar.copy(
                                        ci[:, c, (b + 1) * BC:(b + 2) * BC],
                                        hbt_n[:, c, :])
                            hbt = hbt_n

                        # ---- Phase C: corrected scan + outputs --------------
                        with tc.tile_pool(name="osb", bufs=2) as osb_p:
                            _mark(nc, "C")
                            st_prev = ci
                            for j in range(K):
                                xps = xps_p.tile([P, UC, R_LANES], FP32, tag="xps")
                                nc.sync.dma_start(out=xps, in_=xp_v[:, :, j, :])
                                st_new = st_p.tile([P, UC, R_LANES], BF16, tag="st")
                                for m in range(UC):
                                    ps = psA.tile([P, R_LANES], FP32)
                                    for k in range(UC):
                                        nc.tensor.matmul(
                                            ps, r_sb[:, k, m * P:(m + 1) * P],
                                            st_prev[:, k, :],
                                            start=(k == 0), stop=(k == UC - 1),
                                        )
                                    nc.vector.tensor_add(
                                        st_new[:, m, :], ps, xps[:, m, :])
                                st_prev = st_new
                                # bounce transposed bf16 state through DRAM,
                                # read back transposed via DMA xbar
                                nc.sync.dma_start(
                                    out=osc_v[:, :, j, :], in_=st_new)
                                for h in range(2):
                                    w = 2 * j + h
                                    tb = osb_p.tile([P, U], BF16, tag="tb")
                                    nc.sync.dma_start(
                                        out=tb,
                                        in_=osc_flat[:, w * P:(w + 1) * P],
                                        transpose=True)
                                    osb = osb_p.tile([P, U], FP32, tag="osb")
                                    nc.vector.tensor_copy(osb, tb)
                                    nc.sync.dma_start(
                                        out=out4[h * 16:(h + 1) * 16, :, j, :],
                                        in_=osb,
                                    )

        _mark(nc, "end")
        with tc.tile_pool(name="repsp", bufs=1) as reps_p:
            rtile = reps_p.tile([1, 1], mybir.dt.int32)
            nc.sync.dma_start(out=rtile, in_=reps_in)
            reps_val = nc.values_load(rtile[0:1, 0:1])
            with tc.For_i(0, reps_val, 1) as _it:
                _body(_it)

    nc.compile()
    return nc


_NC_CACHE = {}


def _get_nc(reps=1):
    if "nc" not in _NC_CACHE:
        _NC_CACHE["nc"] = build_nc()
    return _NC_CACHE["nc"]


def _make_in_maps(sequences, kernel, recurrent_kernel, h0, reps=1):
    in_maps = []
    for c in range(NCORES):
        sl = slice(c * BC, (c + 1) * BC)
        in_maps.append({
            "seq": sequences[sl],
            "w": kernel,
            "r": recurrent_kernel,
            "h0": h0[sl],
            "reps": np.array([[reps]], dtype=np.int32),
        })
    return in_maps


def bench(inputs, reps):
    from concourse.bass_utils import run_bass_kernel_spmd
    nc = _get_nc()
    in_maps = _make_in_maps(
        np.ascontiguousarray(inputs["sequences"], dtype=np.float32),
        np.ascontiguousarray(inputs["kernel"], dtype=np.float32),
        np.ascontiguousarray(inputs["recurrent_kernel"], dtype=np.float32),
        np.ascontiguousarray(inputs["h0"], dtype=np.float32), reps)
    return run_bass_kernel_spmd(nc, in_maps, core_ids=list(range(NCORES)))


def kernel(sequences, kernel, recurrent_kernel, h0):
    from concourse.bass_utils import run_bass_kernel_spmd
    nc = _get_nc()
    sequences = np.ascontiguousarray(sequences, dtype=np.float32)
    kernel = np.ascontiguousarray(kernel, dtype=np.float32)
    recurrent_kernel = np.ascontiguousarray(recurrent_kernel, dtype=np.float32)
    h0 = np.ascontiguousarray(h0, dtype=np.float32)
    in_maps = _make_in_maps(sequences, kernel, recurrent_kernel, h0)
    res = run_bass_kernel_spmd(nc, in_maps, core_ids=list(range(NCORES)))
    return np.concatenate([res.results[c]["out"] for c in range(NCORES)], axis=0)


# ---------------------------------------------------------------- dev tools
def _numpy_model(seqs, W, R, h0):
    """Blocked-scan numpy model (per-core shard shapes)."""
    xp = seqs.reshape(-1, F) @ W
    xp = xp.reshape(seqs.shape[0], T, U)
    h = h0.copy()
    outs = np.zeros((seqs.shape[0], T, U), np.float32)
    for t in range(T):
        h = xp[:, t] + h @ R
        outs[:, t] = h
    return outs


def _selftest_sim():
    from concourse.bass_interp import CoreSim
    rng = np.random.default_rng(1)
    seqs = rng.standard_normal((BC, T, F), dtype=np.float32)
    W = (rng.standard_normal((F, U)) * 0.02).astype(np.float32)
    R = (rng.standard_normal((U, U)) * 0.02).astype(np.float32)
    h0 = np.zeros((BC, U), np.float32)
    nc = _get_nc()
    sim = CoreSim(nc, trace=False)
    sim.tensor("seq")[:] = seqs
    sim.tensor("w")[:] = W
    sim.tensor("r")[:] = R
    sim.tensor("h0")[:] = h0
    sim.tensor("reps")[:] = np.array([[1]], dtype=np.int32)
    sim.simulate(check_with_hw=False)
    got = np.asarray(sim.tensor("out"))
    exp = _numpy_model(seqs, W, R, h0)
    err = np.abs(got - exp).max() / np.abs(exp).max()
    print("sim relerr:", err)


def _selftest_hw():
    rng = np.random.default_rng(1)
    seqs = rng.standard_normal((B, T, F), dtype=np.float32)
    W = (rng.standard_normal((F, U)) * 0.02).astype(np.float32)
    R = (rng.standard_normal((U, U)) * 0.02).astype(np.float32)
    h0 = np.zeros((B, U), np.float32)
    got = kernel(seqs, W, R, h0)
    exp = _numpy_model(seqs, W, R, h0)
    err = np.abs(got - exp).max() / np.abs(exp).max()
    print("hw relerr:", err)


if __name__ == "__main__":
    if len(sys.argv) > 1 and sys.argv[1] == "sim":
        _selftest_sim()
    else:
        _selftest_hw()



# revision 2
# speedup vs baseline: 189.2833x; 189.2833x over previous
"""Trainium2 Bass kernel for nn_RNN: h_t = x_t @ W + h_{t-1} @ R (linear RNN). v2.

Full shapes: sequences [64, 512, 1024], kernel [1024, 1024],
recurrent_kernel [1024, 1024], h0 [64, 1024] -> out [64, 512, 1024].
Sharding: data-parallel over batch across 8 cores (8 sequences/core).

Per-core blocked scan, K=16 block length, NB=32 blocks, lane r = blk*8+b.
v2 changes vs v1:
  - xproj kept SBUF-resident (bf16, 64KB/partition); no DRAM xp bounce.
  - X transposes via DMA xbar (seq cast to bf16 time-major in DRAM by a
    SWDGE cast DMA, then hardware DMA-transpose loads), not PE.
  - squaring-chain transposes via DRAM bf16 bounce + DMA-transpose, not PE.
  - output path: osc bf16 bounce (ACT queue) -> DMA-transpose (SP queue)
    -> SWDGE cast bf16->fp32 store (Pool queue); no DVE copy.
"""
import sys
import numpy as np

sys.path.insert(0, "/opt/trn_rl_repo")

try:  # persistent jit cache: repeated kernel() invocations skip recompile
    import jax
    import os as _os
    _cache = _os.environ.get("JAX_COMPILATION_CACHE_DIR", "/tmp/jaxcache_rnn")
    _os.makedirs(_cache, exist_ok=True)
    jax.config.update("jax_compilation_cache_dir", _cache)
except Exception:
    pass

import concourse.bass as bass  # noqa: E402
import concourse.tile as tile  # noqa: E402
from concourse import bacc, mybir  # noqa: E402
from concourse.masks import make_identity  # noqa: E402

FP32 = mybir.dt.float32
BF16 = mybir.dt.bfloat16

NCORES = 8
B, T, F, U = 64, 512, 1024, 1024
BC = B // NCORES          # batch per core = 8
K = 16                    # block length
NB = T // K               # 32 blocks
R_LANES = NB * BC         # 256 scan lanes
P = 128                   # partitions
FC = F // P               # 8 f-chunks
UC = U // P               # 8 u-chunks


def build_nc():
    nc = bacc.Bacc("TRN2", target_bir_lowering=False, debug=False,
                   num_devices=NCORES)

    seq = nc.dram_tensor("seq", [BC, T, F], FP32, kind="ExternalInput").ap()
    w_in = nc.dram_tensor("w", [F, U], FP32, kind="ExternalInput").ap()
    r_in = nc.dram_tensor("r", [U, U], FP32, kind="ExternalInput").ap()
    h0_in = nc.dram_tensor("h0", [BC, U], FP32, kind="ExternalInput").ap()
    reps_in = nc.dram_tensor("reps", [1, 1], mybir.dt.int32,
                             kind="ExternalInput").ap()
    out = nc.dram_tensor("out", [BC, T, U], FP32, kind="ExternalOutput").ap()
    # bf16 copy of seq, w-major per jj-chunk: xbf[jj, w, f], w = blk*16+j2*8+b
    # (t = blk*16 + jj*2 + j2)
    xbf = nc.dram_tensor("xbf", [8, 512, F], BF16).ap()
    # squaring-chain transpose bounce: pw[i] holds R^(2^i) natural (bf16)
    pw = nc.dram_tensor("pw", [4, U, U], BF16).ap()
    # output bounce scratch (bf16, transposed [u, j, r])
    osc_d = nc.dram_tensor("out_scratch", [U, K, R_LANES], BF16).ap()

    # DRAM views
    seq5 = seq.rearrange("b (blk jj j2) f -> jj blk j2 b f", blk=NB, jj=8, j2=2)
    xbf_w = xbf.rearrange("jj (blk j2 b) f -> jj blk j2 b f", blk=NB, j2=2)
    osc_v = osc_d.rearrange("(m p) j r -> p m j r", p=P)
    osc_flat = osc_d.rearrange("u j r -> u (j r)")           # [1024, 4096]
    out4 = out.rearrange("b (nb k) u -> nb b k u", k=K)      # [32, 8, 16, 1024]

    with tile.TileContext(nc) as tc:
        def _body(_it=None):
            with (
                tc.tile_pool(name="consts", bufs=1) as consts,
                tc.tile_pool(name="mats", bufs=1) as mats,
                tc.tile_pool(name="xpsb", bufs=1) as xpsb_p,
                tc.tile_pool(name="psA", bufs=4, space="PSUM") as psA,
                tc.tile_pool(name="psT", bufs=2, space="PSUM") as psT,
            ):
                id8 = consts.tile([BC, BC], FP32)
                make_identity(nc, id8)

                # ---- load W, R (cast fp32->bf16 during DMA) ----------------
                # mats pool rotating slots: r, s0 (transposed power), s1, s2.
                # W shares slot s2 (dead after xproj, before s2's first write
                # at R^4 -- the WAR dep orders R^4 after X's last matmul).
                w_sb = mats.tile([P, FC, U], BF16, tag="s2")
                r_sb = mats.tile([P, UC, U], BF16, tag="r")
                nc.gpsimd.dma_start(
                    out=w_sb, in_=w_in.rearrange("(k p) u -> p k u", p=P))
                nc.gpsimd.dma_start(
                    out=r_sb, in_=r_in.rearrange("(k p) u -> p k u", p=P))

                # ---- X input cast: seq fp32 -> xbf bf16 (w-major) ----------
                # split per (jj, j2): DMA AP balancing is limited to 3 dims
                for jj in range(8):
                    for j2 in range(2):
                        nc.gpsimd.dma_start(out=xbf_w[jj, :, j2],
                                            in_=seq5[jj, :, j2])

                # ---- Phase X: xproj -> xpsb (SBUF-resident, bf16) ----------
                xpsb = xpsb_p.tile([P, UC, K, R_LANES], BF16)
                with tc.tile_pool(name="xt", bufs=2) as xt_p:
                    for jj in range(8):
                        xt = xt_p.tile([P, FC, 512], BF16, tag="xt")
                        for c in range(FC):
                            nc.sync.dma_start(
                                out=xt[:, c, :],
                                in_=xbf[jj][:, c * P:(c + 1) * P],
                                transpose=True)
                        for m in range(UC):
                            ps = psA.tile([P, 512], FP32)
                            for k in range(FC):
                                nc.tensor.matmul(
                                    ps, w_sb[:, k, m * P:(m + 1) * P],
                                    xt[:, k, :],
                                    start=(k == 0), stop=(k == FC - 1),
                                )
                            # scatter psum (blk,j2,b) -> xpsb[(j2),(blk,b)]
                            nc.vector.tensor_copy(
                                xpsb[:, m, 2 * jj:2 * jj + 2, :].rearrange(
                                    "p j (blk b) -> p j blk b", blk=NB),
                                ps.rearrange("p (blk j2 b) -> p j2 blk b",
                                             blk=NB, j2=2, b=BC),
                            )

                # ---- Phase S: R^16 via squarings; transposes via DMA -------
                def gemm1024(dst, lhsT_t, rhs_t):
                    for m in range(UC):
                        for n in range(2):
                            ps = psA.tile([P, 512], FP32)
                            for k in range(UC):
                                nc.tensor.matmul(
                                    ps, lhsT_t[:, k, m * P:(m + 1) * P],
                                    rhs_t[:, k, n * 512:(n + 1) * 512],
                                    start=(k == 0), stop=(k == UC - 1),
                                )
                            nc.vector.tensor_copy(
                                dst[:, m, n * 512:(n + 1) * 512], ps)

                def dma_transpose1024(dst, src_sb, slot):
                    # store src (tiled [p,k,u] bf16) -> pw[slot] natural, then
                    # xbar-transpose-read back into dst (tiled [p,k,u] = src.T)
                    nc.scalar.dma_start(
                        out=pw[slot].rearrange("(k p) u -> p k u", p=P),
                        in_=src_sb)
                    for c in range(UC):
                        nc.sync.dma_start(
                            out=dst[:, c, :],
                            in_=pw[slot][:, c * P:(c + 1) * P],
                            transpose=True)

                rt_t = mats.tile([P, UC, U], BF16, tag="s0")
                dma_transpose1024(rt_t, r_sb, 0)
                r2 = mats.tile([P, UC, U], BF16, tag="s1")
                gemm1024(r2, rt_t, r_sb)                  # R^2
                r2t = mats.tile([P, UC, U], BF16, tag="s0")
                dma_transpose1024(r2t, r2, 1)
                r4 = mats.tile([P, UC, U], BF16, tag="s2")
                gemm1024(r4, r2t, r2)                     # R^4
                r4t = mats.tile([P, UC, U], BF16, tag="s0")
                dma_transpose1024(r4t, r4, 2)
                r8 = mats.tile([P, UC, U], BF16, tag="s1")
                gemm1024(r8, r4t, r4)                     # R^8
                r8t = mats.tile([P, UC, U], BF16, tag="s0")
                dma_transpose1024(r8t, r8, 3)
                r16 = mats.tile([P, UC, U], BF16, tag="s2")
                gemm1024(r16, r8t, r8)                    # R^16

                # ---- Phase A: zero-init batched scan -----------------------
                with (
                    tc.tile_pool(name="st", bufs=2) as st_p,
                    tc.tile_pool(name="psb", bufs=1) as psb_p,
                ):
                    st_prev = xpsb[:, :, 0, :]
                    for j in range(1, K):
                        st_new = st_p.tile([P, UC, R_LANES], BF16, tag="st")
                        for m in range(UC):
                            ps = psA.tile([P, R_LANES], FP32)
                            for k in range(UC):
                                nc.tensor.matmul(
                                    ps, r_sb[:, k, m * P:(m + 1) * P],
                                    st_prev[:, k, :],
                                    start=(k == 0), stop=(k == UC - 1),
                                )
                            nc.vector.tensor_add(st_new[:, m, :], ps,
                                                 xpsb[:, m, j, :])
                        st_prev = st_new
                    psb = psb_p.tile([P, UC, R_LANES], FP32)
                    nc.scalar.copy(psb, st_prev)

                    # ---- Phase B: boundary scan (32 blocks, thin) ----------
                    with (
                        tc.tile_pool(name="hbt", bufs=2) as hbt_p,
                        tc.tile_pool(name="ci", bufs=1) as ci_p,
                        tc.tile_pool(name="mn", bufs=2) as mn_p,
                    ):
                        ci = ci_p.tile([P, UC, R_LANES], BF16)
                        h0sb = mn_p.tile([BC, U], FP32, tag="h0")
                        nc.sync.dma_start(out=h0sb, in_=h0_in)
                        hbt = hbt_p.tile([P, UC, BC], BF16, tag="hbt")
                        for c in range(UC):
                            pt = psT.tile([P, BC], FP32)
                            nc.tensor.transpose(pt, h0sb[:, c * P:(c + 1) * P], id8)
                            nc.scalar.copy(hbt[:, c, :], pt)
                            nc.scalar.copy(ci[:, c, 0:BC], pt)
                        for b in range(NB):
                            mn = mn_p.tile([BC, U], FP32, tag="mn")
                            for n in range(2):
                                ps = psA.tile([BC, 512], FP32)
                                for k in range(UC):
                                    nc.tensor.matmul(
                                        ps, hbt[:, k, :],
                                        r16[:, k, n * 512:(n + 1) * 512],
                                        start=(k == 0), stop=(k == UC - 1),
                                    )
                                nc.scalar.copy(mn[:, n * 512:(n + 1) * 512], ps)
                            hbt_n = hbt_p.tile([P, UC, BC], BF16, tag="hbt")
                            for c in range(UC):
                                pt = psT.tile([P, BC], FP32)
                                nc.tensor.transpose(pt, mn[:, c * P:(c + 1) * P], id8)
                                nc.vector.tensor_add(
                                    hbt_n[:, c, :], pt, psb[:, c, b * BC:(b + 1) * BC])
                                if b < NB - 1:
                                    nc.scalar.copy(
                                        ci[:, c, (b + 1) * BC:(b + 2) * BC],
                                        hbt_n[:, c, :])
                            hbt = hbt_n

                        # ---- Phase C: corrected scan + outputs -------------
                        # per j: st -> osc (DRAM bf16) -> DMA-transpose into
                        # out_acc [lane, jg, u]; after each 8-j group, one big
                        # cast-store per lane-half (64KB-contiguous runs).
                        with tc.tile_pool(name="oacc", bufs=1) as oacc_p:
                            st_prev = ci
                            for g in range(2):
                                oacc = oacc_p.tile([P, 2, K // 2, U], BF16,
                                                   tag="oacc")
                                for jg in range(K // 2):
                                    j = g * (K // 2) + jg
                                    st_new = st_p.tile([P, UC, R_LANES], BF16,
                                                       tag="st")
                                    for m in range(UC):
                                        ps = psA.tile([P, R_LANES], FP32)
                                        for k in range(UC):
                                            nc.tensor.matmul(
                                                ps, r_sb[:, k, m * P:(m + 1) * P],
                                                st_prev[:, k, :],
                                                start=(k == 0), stop=(k == UC - 1),
                                            )
                                        nc.vector.tensor_add(
                                            st_new[:, m, :], ps, xpsb[:, m, j, :])
                                    st_prev = st_new
                                    nc.scalar.dma_start(
                                        out=osc_v[:, :, j, :], in_=st_new)
                                    for h in range(2):
                                        w = 2 * j + h
                                        nc.sync.dma_start(
                                            out=oacc[:, h, jg, :],
                                            in_=osc_flat[:, w * P:(w + 1) * P],
                                            transpose=True)
                                for h in range(2):
                                    nc.gpsimd.dma_start(
                                        out=out4[h * 16:(h + 1) * 16, :,
                                                 g * (K // 2):(g + 1) * (K // 2), :],
                                        in_=oacc[:, h, :, :],
                                    )

        with tc.tile_pool(name="repsp", bufs=1) as reps_p:
            rtile = reps_p.tile([1, 1], mybir.dt.int32)
            nc.sync.dma_start(out=rtile, in_=reps_in)
            reps_val = nc.values_load(rtile[0:1, 0:1])
            with tc.For_i(0, reps_val, 1) as _it:
                _body(_it)

    nc.compile()
    return nc


_NC_CACHE = {}


def _get_nc(reps=1):
    if "nc" not in _NC_CACHE:
        _NC_CACHE["nc"] = build_nc()
    return _NC_CACHE["nc"]


def _make_in_maps(sequences, kernel, recurrent_kernel, h0, reps=1):
    in_maps = []
    for c in range(NCORES):
        sl = slice(c * BC, (c + 1) * BC)
        in_maps.append({
            "seq": sequences[sl],
            "w": kernel,
            "r": recurrent_kernel,
            "h0": h0[sl],
            "reps": np.array([[reps]], dtype=np.int32),
        })
    return in_maps


def bench(inputs, reps):
    from concourse.bass_utils import run_bass_kernel_spmd
    nc = _get_nc()
    in_maps = _make_in_maps(
        np.ascontiguousarray(inputs["sequences"], dtype=np.float32),
        np.ascontiguousarray(inputs["kernel"], dtype=np.float32),
        np.ascontiguousarray(inputs["recurrent_kernel"], dtype=np.float32),
        np.ascontiguousarray(inputs["h0"], dtype=np.float32), reps)
    return run_bass_kernel_spmd(nc, in_maps, core_ids=list(range(NCORES)))


def kernel(sequences, kernel, recurrent_kernel, h0):
    from concourse.bass_utils import run_bass_kernel_spmd
    nc = _get_nc()
    sequences = np.ascontiguousarray(sequences, dtype=np.float32)
    kernel = np.ascontiguousarray(kernel, dtype=np.float32)
    recurrent_kernel = np.ascontiguousarray(recurrent_kernel, dtype=np.float32)
    h0 = np.ascontiguousarray(h0, dtype=np.float32)
    in_maps = _make_in_maps(sequences, kernel, recurrent_kernel, h0)
    res = run_bass_kernel_spmd(nc, in_maps, core_ids=list(range(NCORES)))
    return np.concatenate([res.results[c]["out"] for c in range(NCORES)], axis=0)


# ---------------------------------------------------------------- dev tools
def _numpy_model(seqs, W, R, h0):
    xp = seqs.reshape(-1, F) @ W
    xp = xp.reshape(seqs.shape[0], T, U)
    h = h0.copy()
    outs = np.zeros((seqs.shape[0], T, U), np.float32)
    for t in range(T):
        h = xp[:, t] + h @ R
        outs[:, t] = h
    return outs
